# revision 8
# baseline (speedup 1.0000x reference)
"""Trainium2 Bass kernel for nn_AttentionHead_5583457485447 (sparse_attention).

Reference computation (per batch b):
    q = X @ Wq; k = X @ Wk                      # [N, DK]
    s = relu((q @ k.T) / sqrt(DK)) * M_mask     # [N, N]
    out = s @ Z @ Wv                            # [N, DV]

Strategy (8 NeuronCores, data-parallel over batch B=8, one batch per core):
  - Mask is uint8-quantized on host (round(m*255)); the 1/(255*sqrt(DK))
    scale folds into Wv.  Mask DMA traffic halves vs bf16, so the full
    mask stream lands by ~1/3 of the kernel and the PE never starves.
  - Fold Wv into Z on device (ZW = Z @ Wv'); the K=1 tail row of the
    257-row contraction is peeled off as a rank-1 outer-product update
    done on DVE/GpSimd scalar_tensor_tensor (saves 16x257 PE cycles).
  - Scores computed directly in transposed [m, n] layout (lhsT = kT,
    rhs = qT).  Combined projection [Wq|Wk] / [Wk|Wq] gives qkT/kqT
    tiles whose row halves let the two score matmuls of an (even, odd)
    m-tile pair run in PE row groups 0-63 / 64-127 concurrently.
  - Emission order keeps the PE streaming continuously (avoids HAM
    down-clock): warmup -> proj -> zw -> score pairs -> C half-A ->
    C half-B.  C accumulates over m in two 8-matmul PSUM chain passes;
    half-A spills f32 partials, half-B adds them back and stores.
  - relu + u8-mask multiply rotated across ACT/DVE/GpSimd; all matmul
    inputs bf16, accumulation fp32.  Output written [128, 16*257]
    column-tiled (multi-KB DMA descriptors), de-interleaved on host.
"""

import json
import os
import sys

import numpy as np

B, N, D, DK = 8, 2048, 256, 64
DV = D + 1  # 257
NT = N // 128  # 16 tiles along n and along m
PW = 512  # scores matmul moving width
SW = 1024  # elementwise unit width (2 psum banks)
NWARM = 8

LAST_EXEC_NS = None
_CACHE = {}


# --------------------------------------------------------------------------
# Patch 1: this container's walrus build rejects instructions carrying more
# than one semaphore wait. Split excess waits onto same-engine NOPs at the
# serialized-BIR level (generic, covers Tile's drains and compute ops).
# --------------------------------------------------------------------------
def _split_waits_in_bir(bir_json: bytes) -> bytes:
    bir = json.loads(bir_json)
    changed = False
    drop_ldw = os.environ.get("KERNEL_DROP_LDW", "0") == "1"
    for fn in bir.get("functions", []):
        for bb in fn.get("blocks", []):
            insts = bb.get("instructions", [])
            if drop_ldw:
                merged = []
                pend = {}
                for inst in insts:
                    if inst.get("opcode") == "Ldweights":
                        si = inst.get("sync_info") or {}
                        if si.get("on_wait") or si.get("on_update"):
                            pend.setdefault(inst["engine"], []).append(si)
                        changed = True
                        continue
                    if inst.get("opcode") == "Matmult" and pend.get(inst.get("engine")):
                        tgt = inst.setdefault("sync_info", {"on_update": [], "on_wait": []})
                        tgt.setdefault("on_wait", [])
                        tgt.setdefault("on_update", [])
                        for si in pend.pop(inst["engine"]):
                            tgt["on_wait"] += si.get("on_wait") or []
                            tgt["on_update"] += si.get("on_update") or []
                    merged.append(inst)
                insts = merged
            out = []
            for inst in insts:
                si = inst.get("sync_info")
                ow = (si or {}).get("on_wait") or []
                if len(ow) > 1:
                    changed = True
                    for i, w in enumerate(ow[:-1]):
                        out.append({
                            "debug": inst.get("debug", 0),
                            "engine": inst["engine"],
                            "ins": [],
                            "name": f"{inst['name']}-ws{i}",
                            "opcode": "NoOp",
                            "outs": [],
                            "sync_info": {"on_update": [], "on_wait": [w]},
                            "text_hint": "wait_split",
                        })
                    si["on_wait"] = [ow[-1]]
                out.append(inst)
            bb["instructions"] = out
    return json.dumps(bir).encode() if changed else bir_json


def _apply_bir_patch():
    import concourse.bass_utils as bass_utils
    import concourse.bass2jax as bass2jax

    if os.environ.get("KERNEL_LDW_OPT", "0") == "1":
        rc_orig = bass_utils.run_command
        if not getattr(rc_orig, "_ldw_wrapped", False):
            def rc_wrapped(argv, **kwargs):
                argv = [a.replace("--enable-ldw-opt=false", "--enable-ldw-opt=true")
                        if isinstance(a, str) else a for a in argv]
                return rc_orig(argv, **kwargs)
            rc_wrapped._ldw_wrapped = True
            bass_utils.run_command = rc_wrapped

    orig = bass_utils.compile_bir_kernel
    if getattr(orig, "_wait_split_wrapped", False):
        return

    def wrapped(bir_json, tmpdir, neff_name="file.neff"):
        if isinstance(bir_json, str):
            bir_json = bir_json.encode()
        return orig(_split_waits_in_bir(bir_json), tmpdir, neff_name=neff_name)

    wrapped._wait_split_wrapped = True
    bass_utils.compile_bir_kernel = wrapped
    bass2jax.compile_bir_kernel = wrapped


# --------------------------------------------------------------------------
# Patch 2: optional NTFF profiling hook for axon (exec-time measurement).
# --------------------------------------------------------------------------
def _install_profile_shim():
    import types, ctypes, contextlib

    if "antenv.axon_hooks" in sys.modules:
        return
    so_path = "/opt/axon/libaxon_pjrt.so"
    if not os.path.exists(so_path):
        return
    lib = ctypes.CDLL(so_path)
    if not hasattr(lib, "axon_start_nrt_profile"):
        return
    lib.axon_start_nrt_profile.argtypes = [ctypes.POINTER(ctypes.c_int64), ctypes.c_size_t]
    lib.axon_start_nrt_profile.restype = ctypes.c_int64
    lib.axon_stop_nrt_profile.argtypes = [ctypes.c_char_p]
    lib.axon_stop_nrt_profile.restype = ctypes.c_int64

    @contextlib.contextmanager
    def _hook(output_dir, device_ids):
        import jax

        jax.devices()
        if device_ids:
            ids = (ctypes.c_int64 * len(device_ids))(*device_ids)
            rc = lib.axon_start_nrt_profile(ids, len(device_ids))
        else:
            rc = lib.axon_start_nrt_profile(None, 0)
        if rc != 0:
            raise RuntimeError(f"axon_start_nrt_profile rc={rc}")
        try:
            yield
        finally:
            n = lib.axon_stop_nrt_profile(str(output_dir).encode())
            print(f"profile: {n} file(s) written to {output_dir}", file=sys.stderr)

    mod = types.ModuleType("antenv.axon_hooks")
    mod.get_axon_ntff_profile_hook = lambda: _hook
    sys.modules["antenv.axon_hooks"] = mod


# --------------------------------------------------------------------------
# Device program (identical for all 8 cores; one batch per core)
# --------------------------------------------------------------------------
def _build_nc():
    import concourse.bass as bass
    import concourse.mybir as mybir
    import concourse.tile as tile

    f32 = mybir.dt.float32
    bf16 = mybir.dt.bfloat16
    u8 = mybir.dt.uint8
    Alu = mybir.AluOpType
    Act = mybir.ActivationFunctionType

    nc = bass.Bass("TRN2", debug=False)

    # mask pre-paired on host: pair p rows 128p..128p+127, cols [blk 2p | blk 2p+1]
    d_mask = nc.dram_tensor("maskp", [N // 2, 2 * N], u8, kind="ExternalInput")
    d_X = nc.dram_tensor("Xp", [128, 2 * N], bf16, kind="ExternalInput")
    # Z packed: chunk0 | chunk1 (rows 0..255 of Z^T)
    d_Z = nc.dram_tensor("Zp", [128, 2 * N], bf16, kind="ExternalInput")
    # W packed: Wqk(256) | Wkq(256) | Wv2(514)
    d_W = nc.dram_tensor("Wp", [128, 1026], bf16, kind="ExternalInput")
    # aux: row 256 of Z^T (N cols) | row 256 of Wv' (DV cols)
    d_aux = nc.dram_tensor("auxp", [1, N + DV], bf16, kind="ExternalInput")
    d_out = nc.dram_tensor("out", [128, NT * DV], f32, kind="ExternalOutput")

    with tile.TileContext(nc) as tc:
        with (
            tc.tile_pool(name="wts", bufs=1) as wts,          # weights/X/Z/qkT/zw
            tc.tile_pool(name="maskp", bufs=8) as maskp,      # mask pairs [128, 4096] u8
            tc.tile_pool(name="mskd", bufs=NT) as mskdp,      # persistent masked tiles
            tc.tile_pool(name="pAp", bufs=NT) as pAp,         # half-A C partials (f32)
            tc.tile_pool(name="rlp", bufs=6) as rlp,          # relu staging (ACT path)
            tc.tile_pool(name="zpp", bufs=4) as zpp,          # zw partial staging (bf16)
            tc.tile_pool(name="outp", bufs=4) as outp,        # out staging
            tc.tile_pool(name="psS", bufs=4, space="PSUM") as psS,  # 4 x 2 banks
        ):
            # ---- input DMAs (only sync/scalar/gpsimd can issue): X halves
            # first on scalar+gpsimd, then Z halves; W + mask + stores on
            # sync. Mask gated behind X only (Z races the mask stream).
            w_sb = wts.tile([128, 1026], bf16, tag="w", name="w")
            nc.sync.dma_start(w_sb[:], d_W.ap()[:, :])
            aux = wts.tile([1, N + DV], bf16, tag="aux", name="aux")
            nc.sync.dma_start(aux[:], d_aux.ap()[:, :])

            wu = wts.tile([128, PW], bf16, tag="wu", name="wu")
            nc.vector.memset(wu[:], 0.0)

            xt = [wts.tile([128, N], bf16, tag=f"xt{c}", name=f"xt{c}") for c in range(2)]
            zt = [
                wts.tile([128, N], bf16, tag="zt0", name="zt0"),
                wts.tile([128, N], bf16, tag="zt1", name="zt1"),
            ]
            nc.scalar.dma_start(xt[0][:], d_X.ap()[:, 0:N])
            nc.gpsimd.dma_start(xt[1][:], d_X.ap()[:, N:2 * N])
            nc.scalar.dma_start(zt[0][:], d_Z.ap()[:, 0:N])
            nc.gpsimd.dma_start(zt[1][:], d_Z.ap()[:, N:2 * N])

            # weight sub-views
            wqk = [w_sb[:, 128 * c:128 * (c + 1)] for c in range(2)]
            wkq = [w_sb[:, 256 + 128 * c:256 + 128 * (c + 1)] for c in range(2)]
            wv = [w_sb[:, 512 + DV * i:512 + DV * (i + 1)] for i in range(2)]
            zrow = aux[0:1, 0:N]        # [1, N]  = Z[:, 256]
            wvrow = aux[0:1, N:N + DV]  # [1, DV] = Wv'[256, :]

            # ---- mask stream: gate on X landing (FIFO queue => whole stream
            # waits), then all 8 pair tiles stream back-to-back ----
            mk = [maskp.tile([128, 2 * N], u8, tag="mask", name=f"mk{p}") for p in range(8)]
            gate = wts.tile([1, 4], bf16, tag="gate", name="gate")
            nc.sync.dma_start(gate[0:1, 0:2], xt[0][0:1, 0:2])
            nc.sync.dma_start(gate[0:1, 2:4], xt[1][0:1, 0:2])
            for p in range(8):
                nc.sync.dma_start(mk[p][:], d_mask.ap()[128 * p:128 * (p + 1), :])

            # ---- PE warm-up: dummy matmuls engage the HAM clock un-throttle
            # while the first DMAs stream in. ----
            for w in range(NWARM):
                pw = psS.tile([128, SW], f32, tag="psS", name=f"psw{w}")
                nc.tensor.matmul(pw[:, :PW], wu[:, :128], wu[:], start=True, stop=True)

            # ---- projections: qkT = [q;k], kqT = [k;q] along partitions ----
            qkT = wts.tile([128, N], bf16, tag="qkT", name="qkT")
            kqT = wts.tile([128, N], bf16, tag="kqT", name="kqT")
            for si, (dst, w_ch) in enumerate(((qkT, wqk), (kqT, wkq))):
                for g in range(N // SW):
                    ps = psS.tile([128, SW], f32, tag="psS", name=f"psa{si}_{g}")
                    for h in range(2):
                        for c in range(2):
                            nc.tensor.matmul(
                                ps[:, h * PW:(h + 1) * PW],
                                w_ch[c],
                                xt[c][:, g * SW + h * PW:g * SW + (h + 1) * PW],
                                start=(c == 0),
                                stop=(c == 1),
                            )
                    if g % 2 == 0:
                        nc.vector.tensor_copy(dst[:, g * SW:(g + 1) * SW], ps[:])
                    else:
                        nc.scalar.activation(dst[:, g * SW:(g + 1) * SW], ps[:], Act.Copy)

            # ---- ZW = Z @ Wv' : two K=128 chunks on PE, the K=1 tail row as
            # a rank-1 update fused into the PSUM evac (stt) ----
            zw_sb = {}

            def emit_zw_pair(mt0):
                ps = psS.tile([128, SW], f32, tag="psS", name=f"pzw{mt0}")
                for j, mt in enumerate((mt0, mt0 + 1)):
                    for i in range(3):
                        lhs = (zt[0], zt[1], zrow)[i]
                        rhs = (wv[0], wv[1], wvrow)[i]
                        lsl = lhs[:, mt * 128:(mt + 1) * 128] if i < 2 else lhs[0:1, mt * 128:(mt + 1) * 128]
                        nc.tensor.matmul(
                            ps[:, j * PW:j * PW + DV],
                            lsl,
                            rhs,
                            start=(i == 0),
                            stop=(i == 2),
                        )
                for j, mt in enumerate((mt0, mt0 + 1)):
                    zw = wts.tile([128, DV], bf16, tag=f"zw{mt}", name=f"zw{mt}")
                    psl = ps[:, j * PW:j * PW + DV]
                    if mt % 2 == 0:
                        nc.scalar.activation(zw[:], psl, Act.Copy)
                    else:
                        nc.vector.tensor_copy(zw[:], psl)
                    zw_sb[mt] = zw

            # ---- score pair: masked[mt] = relu(sT)*maskT for mts (2pr,2pr+1) ----
            # unit modes: "v" = DVE fused max*mult from PSUM, "d"/"g" = ACT relu
            # then DVE/GpSimd multiply (SBUF side).
            masked_sb = {}

            def emit_score_pair(pr, pattern):
                mts = (2 * pr, 2 * pr + 1)
                mkt = mk[pr]
                mds = []
                for mt in mts:
                    md = mskdp.tile([128, N], bf16, tag="masked", name=f"md{mt}")
                    mds.append(md)
                    masked_sb[mt] = md
                for u in range(N // SW):
                    pss = []
                    for j, mt in enumerate(mts):
                        ro = DK * j
                        lhs_t = (kqT, qkT)[j]  # rows ro:ro+64 hold kT
                        rhs_t = (qkT, kqT)[j]  # rows ro:ro+64 hold qT
                        ps = psS.tile([128, SW], f32, tag="psS", name=f"pss{mt}_{u}")
                        for h in range(SW // PW):
                            c0 = u * SW + h * PW
                            nc.tensor.matmul(
                                ps[:, h * PW:(h + 1) * PW],
                                lhs_t[ro:ro + DK, mt * 128:(mt + 1) * 128],
                                rhs_t[ro:ro + DK, c0:c0 + PW],
                                start=True,
                                stop=True,
                            )
                        pss.append(ps)
                    for j, mt in enumerate(mts):
                        sl = slice(u * SW, (u + 1) * SW)
                        mksl = mkt[:, j * N + u * SW:j * N + (u + 1) * SW]
                        mode = pattern[2 * u + j]
                        if mode == "v":
                            nc.vector.scalar_tensor_tensor(
                                mds[j][:, sl], pss[j][:], 0.0, mksl,
                                Alu.max, Alu.mult,
                            )
                        else:
                            rl = rlp.tile([128, SW], bf16, tag="rl", name=f"rl{mt}_{u}")
                            nc.scalar.activation(rl[:], pss[j][:], Act.Relu)
                            eng = nc.gpsimd if mode == "g" else nc.vector
                            eng.tensor_mul(mds[j][:, sl], rl[:], mksl)

            partials = {}

            def emit_c_pair(nt0, mt_range, to_pA=False):
                # two 8-matmul chains (nt0, nt0+1) into the two banks of one
                # 2-bank tile, then two narrow evacs (pA spill or add+store)
                ps = psS.tile([128, SW], f32, tag="psS", name=f"psc{nt0}_{mt_range[0]}")
                for j, nt in enumerate((nt0, nt0 + 1)):
                    for i, mt in enumerate(mt_range):
                        nc.tensor.matmul(
                            ps[:, j * PW:j * PW + DV],
                            masked_sb[mt][:, nt * 128:(nt + 1) * 128],
                            zw_sb[mt][:],
                            start=(i == 0),
                            stop=(i == len(mt_range) - 1),
                        )
                if to_pA:
                    for j, nt in enumerate((nt0, nt0 + 1)):
                        pa = pAp.tile([128, DV], f32, tag="pA", name=f"pa{nt}")
                        partials[nt] = pa
                        psl = ps[:, j * PW:j * PW + DV]
                        if nt % 4 == 3:
                            nc.vector.tensor_copy(pa[:], psl)
                        else:
                            nc.scalar.activation(pa[:], psl, Act.Copy)
                    return
                ot = outp.tile([128, 2 * DV], f32, tag="out", name=f"ot{nt0}")
                for j, nt in enumerate((nt0, nt0 + 1)):
                    psl = ps[:, j * PW:j * PW + DV]
                    nc.vector.tensor_add(ot[:, j * DV:(j + 1) * DV], psl, partials[nt][:])
                nc.sync.dma_start(
                    d_out.ap()[:, nt0 * DV:(nt0 + 2) * DV], ot[:]
                )

            # ---- emission order: keep PE streaming continuously ----
            PAT_A = ("v", "g", "v", "g")
            PAT_B = ("v", "g", "d", "g")

            for mt0 in range(0, NT, 2):
                emit_zw_pair(mt0)
            for pr in range(4):
                emit_score_pair(pr, PAT_A)
            for pr in range(4, 8):
                emit_score_pair(pr, PAT_B)

            # C pass A (mt 0..5, 6-matmul chains) -> spill partials.  The
            # short first pass only needs the first 6 masked tiles, giving
            # the elementwise engines slack; pass B (mt 6..15) starts late
            # enough that the whole mask stream has been processed.
            pass_a = list(range(6))
            for k in range(NT // 2):
                emit_c_pair(2 * k, pass_a, to_pA=True)

            # C pass B (mt 6..15, 10-matmul chains) + partial add + store
            pass_b = list(range(6, NT))
            for k in range(NT // 2):
                emit_c_pair(2 * k, pass_b)

    return nc


def kernel(Z_l, X_l, M_mask, Wq, Wk, Wv):
    global LAST_EXEC_NS
    _apply_bir_patch()

    trace = os.environ.get("KERNEL_TRACE", "0") == "1"
    if trace:
        _install_profile_shim()

    from concourse.bass_utils import run_bass_kernel_spmd

    Z_l = np.asarray(Z_l, dtype=np.float32)
    X_l = np.asarray(X_l, dtype=np.float32)
    M_mask = np.asarray(M_mask, dtype=np.float32)
    Wq = np.asarray(Wq, dtype=np.float32)
    Wk = np.asarray(Wk, dtype=np.float32)
    Wv = np.asarray(Wv, dtype=np.float32)

    import ml_dtypes
    bf = ml_dtypes.bfloat16

    # Host-side layout prep: transposes, bf16 casts, u8 mask quantization,
    # scale folds (1/sqrt(DK) and 1/255 into Wv).
    XT = np.ascontiguousarray(X_l.transpose(0, 2, 1))                # [B, D, N]
    Xp = np.concatenate([XT[:, :128, :], XT[:, 128:, :]], axis=2).astype(bf)  # [B,128,2N]

    ZT = np.ascontiguousarray(Z_l.transpose(0, 2, 1))                # [B, DV, N]
    Zp = np.concatenate(
        [ZT[:, :128, :], ZT[:, 128:256, :]], axis=2
    ).astype(bf)                                                     # [B, 128, 2N]

    MT = M_mask.transpose(0, 2, 1)                                   # [B, N(m), N(n)]
    Mu8 = np.rint(MT * np.float32(255.0)).astype(np.uint8)
    # pair p tile: [128, 2N] = [block 2p | block 2p+1]
    Mp = np.ascontiguousarray(
        Mu8.reshape(B, 8, 2, 128, N).transpose(0, 1, 3, 2, 4).reshape(B, N // 2, 2 * N)
    )

    Wv8 = (Wv / (np.sqrt(np.float32(DK)) * np.float32(255.0))).astype(np.float32)
    Wqk = np.concatenate([Wq, Wk], axis=1)                           # [D, 128]
    Wkq = np.concatenate([Wk, Wq], axis=1)                           # [D, 128]
    Wp = np.concatenate(
        [
            Wqk[:128, :], Wqk[128:, :],          # 256 cols
            Wkq[:128, :], Wkq[128:, :],          # 256 cols
            Wv8[:128, :], Wv8[128:256, :],       # 514 cols
        ],
        axis=1,
    ).astype(bf)                                                     # [128, 1026]
    # aux row: Z^T row 256 (per-batch) | Wv' row 256
    auxp = np.concatenate(
        [ZT[:, 256, :], np.broadcast_to(Wv8[256, :], (B, DV))], axis=1
    ).astype(bf).reshape(B, 1, N + DV)                               # [B, 1, N+DV]

    if "nc" not in _CACHE:
        _CACHE["nc"] = _build_nc()
    nc = _CACHE["nc"]

    in_maps = [
        {
            "maskp": Mp[b],
            "Xp": Xp[b],
            "Zp": Zp[b],
            "Wp": Wp,
            "auxp": auxp[b],
        }
        for b in range(B)
    ]
    try:
        res = run_bass_kernel_spmd(nc, in_maps, core_ids=list(range(B)), trace=trace)
    except Exception:
        # A prior (profiled) run can leave an execution unit wedged; the failed
        # attempt clears it and a retry goes through.
        res = run_bass_kernel_spmd(nc, in_maps, core_ids=list(range(B)), trace=trace)
    _CACHE["last_res"] = res
    if trace:
        LAST_EXEC_NS = res.exec_time_ns
    out = np.stack(
        [
            res.results[b]["out"].reshape(128, NT, DV).transpose(1, 0, 2).reshape(N, DV)
            for b in range(B)
        ],
        axis=0,
    )
    return out
